# revision 11
# baseline (speedup 1.0000x reference)
"""Multi-head attention (B=2, S=2048, RES=1024, H=16) on 8 NeuronCores.

Sharding: batch*heads across cores. Core c handles batch c//4 and heads
4*(c%4) .. 4*(c%4)+3 (column-sharded QKV weights). No cross-core comm.

Per-core kernel (S=2048, K=1024, C=256 = 4 heads x 64), bf16 matmuls
with fp32 PSUM accumulation:
  xT = transpose(x_b)              via PE transpose
  QT = (Wq_c)^T x_b^T  [C, S]      K on partitions
  KT = (Wk_c)^T x_b^T  [C, S]
  V  = x_b Wv_c        [S, C] (+ ones col per head -> softmax sums ride
                               along in the PV matmul; V proj interleaved
                               into the first attention loop)
  per head: scoresT[t,s] = K_h^T Q_h -> exp(x/8) on ACT -> attnT (bf16)
            outT[d,s] (+ sums row) = V_aug^T attnT  (fp32 psum, 16 t-blocks)
            DMA xbar-transpose outT back to [s, d], rows * 1/sums, DMA out.

HAM note: the PE clock-gate un-throttles only under dense full-array
activity; attention's half-array matmuls can leave a throttled core stuck
at 1.2 GHz. The V-proj interleave plus tiny full-array "warm" matmuls
(overwritten by the next QK) keep the issue rate above the gate threshold.
"""

import sys

if "/opt/trn_rl_repo" not in sys.path:
    sys.path.insert(0, "/opt/trn_rl_repo")

import numpy as np

B = 2
S = 2048
RES = 1024
HEADS = 16
HD = 64  # head dim
N_CORES = 8
HPC = 4  # heads per core
C = HPC * HD  # 256 per-core projected width
K = RES  # contraction dim of projections
NKT = K // 128  # 8 k-chunks
NST = S // 128  # 16 s-tiles / t-blocks
SH = 1024  # s-half size for attention inner loop
VAUG = HD + 2  # 66: V cols + ones col + zero pad

_CACHE: dict = {}


def _build_nc():
    import concourse.mybir as mybir
    import concourse.tile as tile
    from concourse import bacc
    from concourse.masks import make_identity

    f32 = mybir.dt.float32
    bf16 = mybir.dt.bfloat16
    AF = mybir.ActivationFunctionType

    nc = bacc.Bacc(None)
    x_in = nc.dram_tensor("x", [S, K], bf16, kind="ExternalInput")
    wq_in = nc.dram_tensor("wq", [K, C], bf16, kind="ExternalInput")
    wk_in = nc.dram_tensor("wk", [K, C], bf16, kind="ExternalInput")
    wv_in = nc.dram_tensor("wv", [K, C], bf16, kind="ExternalInput")
    out_d = nc.dram_tensor("out", [S, C], f32, kind="ExternalOutput")

    with tile.TileContext(nc) as tc:
        with (
            tc.tile_pool(name="persist", bufs=1) as persist,
            tc.tile_pool(name="xw", bufs=1) as xw,
            tc.tile_pool(name="attn", bufs=2) as attn,
        ):
            ident32 = persist.tile([128, 128], f32)
            make_identity(nc, ident32)
            ident = persist.tile([128, 128], bf16)
            nc.vector.tensor_copy(ident[:], ident32[:])
            ones4 = persist.tile([128, HPC], f32)
            nc.vector.memset(ones4[:], 1.0)
            zeros4 = persist.tile([128, HPC], f32)
            nc.vector.memset(zeros4[:], 0.0)

            qt_tiles = []
            kt_tiles = []
            for cb in range(C // 128):
                qt = persist.tile([128, S], bf16, name=f"qt_{cb}", tag="qt", bufs=2)
                kt = persist.tile([128, S], bf16, name=f"kt_{cb}", tag="kt", bufs=2)
                qt_tiles.append(qt)
                kt_tiles.append(kt)

            # V tiles (+ones at col h*VAUG+HD, zero at +HD+1)
            v_aug = []
            for st in range(NST):
                va = persist.tile(
                    [128, HPC * VAUG], bf16, name=f"vaug_{st}", tag="vaug", bufs=NST
                )
                v_aug.append(va)

            out_tiles = []
            for sb in range(NST):
                ot = persist.tile([128, C], f32, name=f"out_{sb}", tag="ot", bufs=NST)
                out_tiles.append(ot)

            # ====== load x, transpose, project Q^T/K^T ======
            with tc.tile_pool(name="ps_pre", bufs=1, space="PSUM") as psp:
                # x^T via DMA xbar transpose straight from DRAM:
                # x[st*128:(st+1)*128, :] (128 x 1024) -> xT3[:, kk, st-cols]
                xT = xw.tile([128, NKT * S], bf16, name="xT")
                xT3 = xT.rearrange("p (k s) -> p k s", k=NKT)
                for st in range(NST):
                    nc.sync.dma_start_transpose(
                        xT3[:, :, st * 128 : (st + 1) * 128],
                        x_in[st * 128 : (st + 1) * 128, :],
                    )

                wq_t = []
                wk_t = []
                wv_t = []
                for kk in range(NKT):
                    wq_kk = xw.tile([128, C], bf16, name=f"wq_{kk}", tag="wq", bufs=NKT)
                    nc.sync.dma_start(wq_kk[:], wq_in[kk * 128 : (kk + 1) * 128, :])
                    wq_t.append(wq_kk)
                    wk_kk = xw.tile([128, C], bf16, name=f"wk_{kk}", tag="wk", bufs=NKT)
                    nc.sync.dma_start(wk_kk[:], wk_in[kk * 128 : (kk + 1) * 128, :])
                    wk_t.append(wk_kk)
                    wv_kk = xw.tile([128, C], bf16, name=f"wv_{kk}", tag="wv", bufs=NKT)
                    nc.sync.dma_start(wv_kk[:], wv_in[kk * 128 : (kk + 1) * 128, :])
                    wv_t.append(wv_kk)

                for w_t, dst in ((wq_t, qt_tiles[0]), (wk_t, kt_tiles[0])):
                    for sc in range(S // 512):
                        pp = psp.tile(
                            [128, 512], f32, name=f"pj0_{sc}", tag="proj",
                            bufs=2,
                        )
                        for kk in range(NKT):
                            nc.tensor.matmul(
                                pp[:],
                                w_t[kk][:, 0:128],
                                xT3[:, kk, sc * 512 : (sc + 1) * 512],
                                start=(kk == 0),
                                stop=(kk == NKT - 1),
                            )
                        nc.vector.tensor_copy(
                            dst[:, sc * 512 : (sc + 1) * 512], pp[:]
                        )

            # ====== attention (V-proj interleaved into first loop) ======
            with tc.tile_pool(name="ps_attn", bufs=1, space="PSUM") as psa:
                tail_groups = []

                def make_vproj(st):
                    def emit():
                        va3 = v_aug[st].rearrange("p (h d) -> p h d", h=HPC)
                        vp = psa.tile(
                            [128, C], f32, name=f"vp_{st}", tag="aux", bufs=2
                        )
                        for kk in range(NKT):
                            nc.tensor.matmul(
                                vp[:],
                                xT3[:, kk, st * 128 : (st + 1) * 128],
                                wv_t[kk][:],
                                start=(kk == 0),
                                stop=(kk == NKT - 1),
                            )
                        nc.vector.tensor_copy(
                            va3[:, :, 0:HD],
                            vp.rearrange("p (h d) -> p h d", h=HPC),
                        )
                        nc.vector.tensor_copy(
                            va3[:, :, HD : HD + 1],
                            ones4.rearrange("p (h o) -> p h o", h=HPC),
                        )
                        nc.vector.tensor_copy(
                            va3[:, :, HD + 1 : HD + 2],
                            zeros4.rearrange("p (h o) -> p h o", h=HPC),
                        )
                    return emit

                proj1_state = {}

                def make_proj1(w_t, dst, sc, half, key):
                    def emit():
                        if half == 0:
                            pp = psa.tile(
                                [128, 512], f32, name=f"pj1_{key}", tag="aux",
                                bufs=2,
                            )
                            proj1_state[key] = pp
                        else:
                            pp = proj1_state.pop(key)
                        for kk in range(half * 4, half * 4 + 4):
                            nc.tensor.matmul(
                                pp[:],
                                w_t[kk][:, 128:256],
                                xT3[:, kk, sc * 512 : (sc + 1) * 512],
                                start=(kk == 0),
                                stop=(kk == NKT - 1),
                            )
                        if half == 1:
                            nc.vector.tensor_copy(
                                dst[:, sc * 512 : (sc + 1) * 512], pp[:]
                            )
                    return emit

                aux_work = [make_vproj(st) for st in range(1, NST)]
                for wi, (w_t, dst) in enumerate(
                    ((wq_t, qt_tiles[1]), (wk_t, kt_tiles[1]))
                ):
                    for sc in range(S // 512):
                        for half in range(2):
                            aux_work.append(
                                make_proj1(w_t, dst, sc, half, f"{wi}_{sc}")
                            )
                # vproj(0) must precede the very first PV
                make_vproj(0)()
                for hp in range(HPC // 2):
                    qt = qt_tiles[hp]
                    kt = kt_tiles[hp]
                    for side in range(2):
                        dlo = side * HD
                        dhi = dlo + HD
                        h_loc = 2 * hp + side
                        for shi in range(S // SH):
                            s0 = shi * SH
                            outp = psa.tile(
                                [VAUG, SH],
                                f32,
                                name=f"outT_{h_loc}_{shi}",
                                tag="outT",
                                bufs=1,
                            )
                            for t in range(NST):
                                had_aux = bool(aux_work)
                                sc_ps = psa.tile(
                                    [128, SH],
                                    f32,
                                    name=f"sc_{h_loc}_{shi}_{t}",
                                    tag="sc",
                                    bufs=2,
                                )
                                if not had_aux:
                                    # tiny full-array matmul, result
                                    # overwritten by QK below (PE clock-gate
                                    # keep-warm; see module docstring)
                                    nc.tensor.matmul(
                                        sc_ps[:, 0:64],
                                        ident[:],
                                        ident[:, 0:64],
                                        start=True,
                                        stop=True,
                                        skip_group_check=True,
                                    )
                                for scj in range(SH // 512):
                                    nc.tensor.matmul(
                                        sc_ps[:, scj * 512 : (scj + 1) * 512],
                                        kt[dlo:dhi, t * 128 : (t + 1) * 128],
                                        qt[
                                            dlo:dhi,
                                            s0 + scj * 512 : s0 + (scj + 1) * 512,
                                        ],
                                        start=True,
                                        stop=True,
                                        skip_group_check=True,
                                    )
                                at = attn.tile(
                                    [128, SH],
                                    bf16,
                                    name=f"at_{h_loc}_{shi}_{t}",
                                    tag="at",
                                    bufs=3,
                                )
                                nc.scalar.activation(
                                    at[:], sc_ps[:], AF.Exp, scale=0.125
                                )
                                for scj in range(SH // 512):
                                    nc.tensor.matmul(
                                        outp[:, scj * 512 : (scj + 1) * 512],
                                        v_aug[t][:, h_loc * VAUG : (h_loc + 1) * VAUG],
                                        at[:, scj * 512 : (scj + 1) * 512],
                                        start=(t == 0),
                                        stop=(t == NST - 1),
                                    )
                                if had_aux:
                                    aux_work.pop(0)()
                            # free psum fast; transpose/normalize deferred
                            oT = attn.tile(
                                [80, SH],
                                bf16,
                                name=f"oT_{h_loc}_{shi}",
                                tag="oT",
                                bufs=8,
                            )
                            nc.vector.tensor_copy(oT[0:VAUG, :], outp[:])
                            tail_groups.append((h_loc, shi, oT))

                # deferred tail: DMA xbar transpose back to [s, d], then
                # normalize rows by 1/sums (col HD of transposed block)
                for h_loc, shi, oT in tail_groups:
                    trb = attn.tile(
                        [128, (SH // 128) * 80],
                        bf16,
                        name=f"trb_{h_loc}_{shi}",
                        tag="trb",
                        bufs=4,
                    )
                    trb3 = trb.rearrange("p (j c) -> p j c", j=SH // 128)
                    nc.sync.dma_start_transpose(trb3[:, :, :], oT[0:80, :])
                    for j in range(SH // 128):
                        sb = shi * (SH // 128) + j
                        rs = attn.tile(
                            [128, 1],
                            f32,
                            name=f"rs_{h_loc}_{shi}_{j}",
                            tag="rs",
                            bufs=8,
                        )
                        nc.vector.reciprocal(rs[:], trb3[:, j, HD : HD + 1])
                        nc.vector.tensor_scalar_mul(
                            out_tiles[sb][:, h_loc * HD : (h_loc + 1) * HD],
                            trb3[:, j, 0:HD],
                            rs[:],
                        )

                for sb in range(NST):
                    nc.sync.dma_start(
                        out_d[sb * 128 : (sb + 1) * 128, :], out_tiles[sb][:]
                    )

    nc.finalize()
    return nc


def _get_nc():
    if "nc" not in _CACHE:
        _CACHE["nc"] = _build_nc()
    return _CACHE["nc"]


def kernel(x, Wq, Wk, Wv):
    import ml_dtypes
    from concourse import bass_utils

    bf = ml_dtypes.bfloat16
    x = np.asarray(x, dtype=np.float32).astype(bf)
    Wq = np.asarray(Wq, dtype=np.float32).astype(bf)
    Wk = np.asarray(Wk, dtype=np.float32).astype(bf)
    Wv = np.asarray(Wv, dtype=np.float32).astype(bf)

    nc = _get_nc()
    in_maps = []
    for c in range(N_CORES):
        b = c // 4
        g = c % 4
        cols = slice(g * C, (g + 1) * C)
        in_maps.append(
            {
                "x": np.ascontiguousarray(x[b]),
                "wq": np.ascontiguousarray(Wq[:, cols]),
                "wk": np.ascontiguousarray(Wk[:, cols]),
                "wv": np.ascontiguousarray(Wv[:, cols]),
            }
        )

    res = bass_utils.run_bass_kernel_spmd(nc, in_maps, list(range(N_CORES)))
    _CACHE["last_results"] = res

    out = np.empty((B, S, RES), dtype=np.float32)
    for c in range(N_CORES):
        b = c // 4
        g = c % 4
        out[b, :, g * C : (g + 1) * C] = res.results[c]["out"]
    return out


# revision 12
# speedup vs baseline: 1.0274x; 1.0274x over previous
"""Multi-head attention (B=2, S=2048, RES=1024, H=16) on 8 NeuronCores.

Sharding: batch*heads across cores. Core c handles batch c//4 and heads
4*(c%4) .. 4*(c%4)+3 (column-sharded QKV weights). No cross-core comm.

Per-core kernel (S=2048, K=1024, C=256 = 4 heads x 64), bf16 matmuls
with fp32 PSUM accumulation:
  xT = transpose(x_b)              via PE transpose
  QT = (Wq_c)^T x_b^T  [C, S]      K on partitions
  KT = (Wk_c)^T x_b^T  [C, S]
  V  = x_b Wv_c        [S, C] (+ ones col per head -> softmax sums ride
                               along in the PV matmul; V proj interleaved
                               into the first attention loop)
  per head: scoresT[t,s] = K_h^T Q_h -> exp(x/8) on ACT -> attnT (bf16)
            outT[d,s] (+ sums row) = V_aug^T attnT  (fp32 psum, 16 t-blocks)
            DMA xbar-transpose outT back to [s, d], rows * 1/sums, DMA out.

HAM note: the PE clock-gate un-throttles only under dense full-array
activity; attention's half-array matmuls can leave a throttled core stuck
at 1.2 GHz. The V-proj interleave plus tiny full-array "warm" matmuls
(overwritten by the next QK) keep the issue rate above the gate threshold.
"""

import sys

if "/opt/trn_rl_repo" not in sys.path:
    sys.path.insert(0, "/opt/trn_rl_repo")

import numpy as np

B = 2
S = 2048
RES = 1024
HEADS = 16
HD = 64  # head dim
N_CORES = 8
HPC = 4  # heads per core
C = HPC * HD  # 256 per-core projected width
K = RES  # contraction dim of projections
NKT = K // 128  # 8 k-chunks
NST = S // 128  # 16 s-tiles / t-blocks
SH = 1024  # s-half size for attention inner loop
VAUG = HD + 2  # 66: V cols + ones col + zero pad

_CACHE: dict = {}


def _build_nc():
    import concourse.mybir as mybir
    import concourse.tile as tile
    from concourse import bacc
    from concourse.masks import make_identity

    f32 = mybir.dt.float32
    bf16 = mybir.dt.bfloat16
    AF = mybir.ActivationFunctionType

    nc = bacc.Bacc(None)
    x_in = nc.dram_tensor("x", [S, K], bf16, kind="ExternalInput")
    wq_in = nc.dram_tensor("wq", [K, C], bf16, kind="ExternalInput")
    wk_in = nc.dram_tensor("wk", [K, C], bf16, kind="ExternalInput")
    wv_in = nc.dram_tensor("wv", [K, C], bf16, kind="ExternalInput")
    out_d = nc.dram_tensor("out", [S, C], f32, kind="ExternalOutput")

    with tile.TileContext(nc) as tc:
        with (
            tc.tile_pool(name="persist", bufs=1) as persist,
            tc.tile_pool(name="xw", bufs=1) as xw,
            tc.tile_pool(name="attn", bufs=2) as attn,
        ):
            ident32 = persist.tile([128, 128], f32)
            make_identity(nc, ident32)
            ident = persist.tile([128, 128], bf16)
            nc.vector.tensor_copy(ident[:], ident32[:])
            ones4 = persist.tile([128, HPC], f32)
            nc.vector.memset(ones4[:], 1.0)
            zeros4 = persist.tile([128, HPC], f32)
            nc.vector.memset(zeros4[:], 0.0)

            qt_tiles = []
            kt_tiles = []
            for cb in range(C // 128):
                qt = persist.tile([128, S], bf16, name=f"qt_{cb}", tag="qt", bufs=2)
                kt = persist.tile([128, S], bf16, name=f"kt_{cb}", tag="kt", bufs=2)
                qt_tiles.append(qt)
                kt_tiles.append(kt)

            # V tiles (+ones at col h*VAUG+HD, zero at +HD+1)
            v_aug = []
            for st in range(NST):
                va = persist.tile(
                    [128, HPC * VAUG], bf16, name=f"vaug_{st}", tag="vaug", bufs=NST
                )
                v_aug.append(va)

            out_tiles = []
            for sb in range(NST):
                ot = persist.tile([128, C], f32, name=f"out_{sb}", tag="ot", bufs=NST)
                out_tiles.append(ot)

            # ====== load x, transpose, project Q^T/K^T ======
            with tc.tile_pool(name="ps_pre", bufs=1, space="PSUM") as psp:
                xT = xw.tile([128, NKT * S], bf16, name="xT")
                xT3 = xT.rearrange("p (k s) -> p k s", k=NKT)
                for st in range(NST):
                    x_t = xw.tile([128, K], bf16, name=f"x_{st}", tag="xload", bufs=3)
                    nc.sync.dma_start(x_t[:], x_in[st * 128 : (st + 1) * 128, :])
                    for kg in range(NKT // 4):
                        tr_ps = psp.tile(
                            [128, 512], bf16, name=f"xtr_{st}_{kg}", tag="xtr", bufs=2
                        )
                        for j in range(4):
                            kk = kg * 4 + j
                            nc.tensor.transpose(
                                tr_ps[:, j * 128 : (j + 1) * 128],
                                x_t[:, kk * 128 : (kk + 1) * 128],
                                ident[:],
                            )
                        nc.vector.tensor_copy(
                            xT3[:, kg * 4 : (kg + 1) * 4, st * 128 : (st + 1) * 128],
                            tr_ps.rearrange("p (j b) -> p j b", j=4),
                        )

                wq_t = []
                wk_t = []
                wv_t = []
                for kk in range(NKT):
                    wq_kk = xw.tile([128, C], bf16, name=f"wq_{kk}", tag="wq", bufs=NKT)
                    nc.sync.dma_start(wq_kk[:], wq_in[kk * 128 : (kk + 1) * 128, :])
                    wq_t.append(wq_kk)
                    wk_kk = xw.tile([128, C], bf16, name=f"wk_{kk}", tag="wk", bufs=NKT)
                    nc.sync.dma_start(wk_kk[:], wk_in[kk * 128 : (kk + 1) * 128, :])
                    wk_t.append(wk_kk)
                    wv_kk = xw.tile([128, C], bf16, name=f"wv_{kk}", tag="wv", bufs=NKT)
                    nc.sync.dma_start(wv_kk[:], wv_in[kk * 128 : (kk + 1) * 128, :])
                    wv_t.append(wv_kk)

                for w_t, dst in ((wq_t, qt_tiles[0]), (wk_t, kt_tiles[0])):
                    for sc in range(S // 512):
                        pp = psp.tile(
                            [128, 512], f32, name=f"pj0_{sc}", tag="proj",
                            bufs=2,
                        )
                        for kk in range(NKT):
                            nc.tensor.matmul(
                                pp[:],
                                w_t[kk][:, 0:128],
                                xT3[:, kk, sc * 512 : (sc + 1) * 512],
                                start=(kk == 0),
                                stop=(kk == NKT - 1),
                            )
                        nc.vector.tensor_copy(
                            dst[:, sc * 512 : (sc + 1) * 512], pp[:]
                        )

            # ====== attention (V-proj interleaved into first loop) ======
            with tc.tile_pool(name="ps_attn", bufs=1, space="PSUM") as psa:
                tail_groups = []

                def make_vproj(st):
                    def emit():
                        va3 = v_aug[st].rearrange("p (h d) -> p h d", h=HPC)
                        vp = psa.tile(
                            [128, C], f32, name=f"vp_{st}", tag="aux", bufs=2
                        )
                        for kk in range(NKT):
                            nc.tensor.matmul(
                                vp[:],
                                xT3[:, kk, st * 128 : (st + 1) * 128],
                                wv_t[kk][:],
                                start=(kk == 0),
                                stop=(kk == NKT - 1),
                            )
                        nc.vector.tensor_copy(
                            va3[:, :, 0:HD],
                            vp.rearrange("p (h d) -> p h d", h=HPC),
                        )
                        nc.vector.tensor_copy(
                            va3[:, :, HD : HD + 1],
                            ones4.rearrange("p (h o) -> p h o", h=HPC),
                        )
                        nc.vector.tensor_copy(
                            va3[:, :, HD + 1 : HD + 2],
                            zeros4.rearrange("p (h o) -> p h o", h=HPC),
                        )
                    return emit

                proj1_state = {}

                def make_proj1(w_t, dst, sc, half, key):
                    def emit():
                        if half == 0:
                            pp = psa.tile(
                                [128, 512], f32, name=f"pj1_{key}", tag="aux",
                                bufs=2,
                            )
                            proj1_state[key] = pp
                        else:
                            pp = proj1_state.pop(key)
                        for kk in range(half * 4, half * 4 + 4):
                            nc.tensor.matmul(
                                pp[:],
                                w_t[kk][:, 128:256],
                                xT3[:, kk, sc * 512 : (sc + 1) * 512],
                                start=(kk == 0),
                                stop=(kk == NKT - 1),
                            )
                        if half == 1:
                            nc.vector.tensor_copy(
                                dst[:, sc * 512 : (sc + 1) * 512], pp[:]
                            )
                    return emit

                aux_work = [make_vproj(st) for st in range(1, NST)]
                for wi, (w_t, dst) in enumerate(
                    ((wq_t, qt_tiles[1]), (wk_t, kt_tiles[1]))
                ):
                    for sc in range(S // 512):
                        for half in range(2):
                            aux_work.append(
                                make_proj1(w_t, dst, sc, half, f"{wi}_{sc}")
                            )
                # vproj(0) must precede the very first PV
                make_vproj(0)()
                for hp in range(HPC // 2):
                    qt = qt_tiles[hp]
                    kt = kt_tiles[hp]
                    for side in range(2):
                        dlo = side * HD
                        dhi = dlo + HD
                        h_loc = 2 * hp + side
                        for shi in range(S // SH):
                            s0 = shi * SH
                            outp = psa.tile(
                                [VAUG, SH],
                                f32,
                                name=f"outT_{h_loc}_{shi}",
                                tag="outT",
                                bufs=1,
                            )
                            for t in range(NST):
                                had_aux = bool(aux_work)
                                sc_ps = psa.tile(
                                    [128, SH],
                                    f32,
                                    name=f"sc_{h_loc}_{shi}_{t}",
                                    tag="sc",
                                    bufs=2,
                                )
                                if not had_aux:
                                    # tiny full-array matmul, result
                                    # overwritten by QK below (PE clock-gate
                                    # keep-warm; see module docstring)
                                    nc.tensor.matmul(
                                        sc_ps[:, 0:64],
                                        ident[:],
                                        ident[:, 0:64],
                                        start=True,
                                        stop=True,
                                        skip_group_check=True,
                                    )
                                for scj in range(SH // 512):
                                    nc.tensor.matmul(
                                        sc_ps[:, scj * 512 : (scj + 1) * 512],
                                        kt[dlo:dhi, t * 128 : (t + 1) * 128],
                                        qt[
                                            dlo:dhi,
                                            s0 + scj * 512 : s0 + (scj + 1) * 512,
                                        ],
                                        start=True,
                                        stop=True,
                                        skip_group_check=True,
                                    )
                                at = attn.tile(
                                    [128, SH],
                                    bf16,
                                    name=f"at_{h_loc}_{shi}_{t}",
                                    tag="at",
                                    bufs=3,
                                )
                                nc.scalar.activation(
                                    at[:], sc_ps[:], AF.Exp, scale=0.125
                                )
                                for scj in range(SH // 512):
                                    nc.tensor.matmul(
                                        outp[:, scj * 512 : (scj + 1) * 512],
                                        v_aug[t][:, h_loc * VAUG : (h_loc + 1) * VAUG],
                                        at[:, scj * 512 : (scj + 1) * 512],
                                        start=(t == 0),
                                        stop=(t == NST - 1),
                                    )
                                if had_aux:
                                    aux_work.pop(0)()
                            # free psum fast; transpose/normalize deferred
                            oT = attn.tile(
                                [80, SH],
                                bf16,
                                name=f"oT_{h_loc}_{shi}",
                                tag="oT",
                                bufs=8,
                            )
                            nc.vector.tensor_copy(oT[0:VAUG, :], outp[:])
                            tail_groups.append((h_loc, shi, oT))

                # deferred tail: DMA xbar transpose back to [s, d], then
                # normalize rows by 1/sums (col HD of transposed block)
                for h_loc, shi, oT in tail_groups:
                    trb = attn.tile(
                        [128, (SH // 128) * 80],
                        bf16,
                        name=f"trb_{h_loc}_{shi}",
                        tag="trb",
                        bufs=4,
                    )
                    trb3 = trb.rearrange("p (j c) -> p j c", j=SH // 128)
                    nc.sync.dma_start_transpose(trb3[:, :, :], oT[0:80, :])
                    for j in range(SH // 128):
                        sb = shi * (SH // 128) + j
                        rs = attn.tile(
                            [128, 1],
                            f32,
                            name=f"rs_{h_loc}_{shi}_{j}",
                            tag="rs",
                            bufs=8,
                        )
                        nc.vector.reciprocal(rs[:], trb3[:, j, HD : HD + 1])
                        nc.vector.tensor_scalar_mul(
                            out_tiles[sb][:, h_loc * HD : (h_loc + 1) * HD],
                            trb3[:, j, 0:HD],
                            rs[:],
                        )

                for sb in range(NST):
                    nc.sync.dma_start(
                        out_d[sb * 128 : (sb + 1) * 128, :], out_tiles[sb][:]
                    )

    nc.finalize()
    return nc


def _get_nc():
    if "nc" not in _CACHE:
        _CACHE["nc"] = _build_nc()
    return _CACHE["nc"]


def kernel(x, Wq, Wk, Wv):
    import ml_dtypes
    from concourse import bass_utils

    bf = ml_dtypes.bfloat16
    x = np.asarray(x, dtype=np.float32).astype(bf)
    Wq = np.asarray(Wq, dtype=np.float32).astype(bf)
    Wk = np.asarray(Wk, dtype=np.float32).astype(bf)
    Wv = np.asarray(Wv, dtype=np.float32).astype(bf)

    nc = _get_nc()
    in_maps = []
    for c in range(N_CORES):
        b = c // 4
        g = c % 4
        cols = slice(g * C, (g + 1) * C)
        in_maps.append(
            {
                "x": np.ascontiguousarray(x[b]),
                "wq": np.ascontiguousarray(Wq[:, cols]),
                "wk": np.ascontiguousarray(Wk[:, cols]),
                "wv": np.ascontiguousarray(Wv[:, cols]),
            }
        )

    res = bass_utils.run_bass_kernel_spmd(nc, in_maps, list(range(N_CORES)))
    _CACHE["last_results"] = res

    out = np.empty((B, S, RES), dtype=np.float32)
    for c in range(N_CORES):
        b = c // 4
        g = c % 4
        out[b, :, g * C : (g + 1) * C] = res.results[c]["out"]
    return out


# revision 13
# speedup vs baseline: 1.2271x; 1.1944x over previous
"""Multi-head attention (B=2, S=2048, RES=1024, H=16) on 8 NeuronCores.

Sharding: batch*heads across cores. Core c handles batch c//4 and heads
4*(c%4) .. 4*(c%4)+3 (column-sharded QKV weights). No cross-core comm.

Per-core kernel (S=2048, K=1024, C=256 = 4 heads x 64), bf16 matmuls
with fp32 PSUM accumulation:
  xT = transpose(x_b)              via PE transpose
  QT = (Wq_c)^T x_b^T  [C, S]      K on partitions
  KT = (Wk_c)^T x_b^T  [C, S]
  V  = x_b Wv_c        [S, C] (+ ones col per head -> softmax sums ride
                               along in the PV matmul; V proj interleaved
                               into the first attention loop)
  per head: scoresT[t,s] = K_h^T Q_h -> exp(x/8) on ACT -> attnT (bf16)
            outT[d,s] (+ sums row) = V_aug^T attnT  (fp32 psum, 16 t-blocks)
            DMA xbar-transpose outT back to [s, d], rows * 1/sums, DMA out.

HAM note: the PE clock-gate un-throttles only under dense full-array
activity; attention's half-array matmuls can leave a throttled core stuck
at 1.2 GHz. The V-proj interleave plus tiny full-array "warm" matmuls
(overwritten by the next QK) keep the issue rate above the gate threshold.
"""

import sys

if "/opt/trn_rl_repo" not in sys.path:
    sys.path.insert(0, "/opt/trn_rl_repo")

import numpy as np

B = 2
S = 2048
RES = 1024
HEADS = 16
HD = 64  # head dim
N_CORES = 8
HPC = 4  # heads per core
C = HPC * HD  # 256 per-core projected width
K = RES  # contraction dim of projections
NKT = K // 128  # 8 k-chunks
NST = S // 128  # 16 s-tiles / t-blocks
SH = 1024  # s-half size for attention inner loop
VAUG = HD + 2  # 66: V cols + ones col + zero pad

_CACHE: dict = {}


def _build_nc():
    import concourse.mybir as mybir
    import concourse.tile as tile
    from concourse import bacc
    from concourse.masks import make_identity

    f32 = mybir.dt.float32
    bf16 = mybir.dt.bfloat16
    AF = mybir.ActivationFunctionType

    nc = bacc.Bacc(None)
    x_in = nc.dram_tensor("x", [S, K], bf16, kind="ExternalInput")
    wq_in = nc.dram_tensor("wq", [K, C], bf16, kind="ExternalInput")
    wk_in = nc.dram_tensor("wk", [K, C], bf16, kind="ExternalInput")
    wv_in = nc.dram_tensor("wv", [K, C], bf16, kind="ExternalInput")
    out_d = nc.dram_tensor("out", [S, C], f32, kind="ExternalOutput")

    with tile.TileContext(nc) as tc:
        with (
            tc.tile_pool(name="persist", bufs=1) as persist,
            tc.tile_pool(name="xw", bufs=1) as xw,
            tc.tile_pool(name="attn", bufs=2) as attn,
        ):
            ident32 = persist.tile([128, 128], f32)
            make_identity(nc, ident32)
            ident = persist.tile([128, 128], bf16)
            nc.vector.tensor_copy(ident[:], ident32[:])
            ones4 = persist.tile([128, HPC], f32)
            nc.vector.memset(ones4[:], 1.0)
            zeros4 = persist.tile([128, HPC], f32)
            nc.vector.memset(zeros4[:], 0.0)

            qt_tiles = []
            kt_tiles = []
            for cb in range(C // 128):
                qt = persist.tile([128, S], bf16, name=f"qt_{cb}", tag="qt", bufs=2)
                kt = persist.tile([128, S], bf16, name=f"kt_{cb}", tag="kt", bufs=2)
                qt_tiles.append(qt)
                kt_tiles.append(kt)

            # V tiles (+ones at col h*VAUG+HD, zero at +HD+1)
            v_aug = []
            for st in range(NST):
                va = persist.tile(
                    [128, HPC * VAUG], bf16, name=f"vaug_{st}", tag="vaug", bufs=NST
                )
                v_aug.append(va)

            out_tiles = []
            for sb in range(NST):
                ot = persist.tile([128, C], f32, name=f"out_{sb}", tag="ot", bufs=NST)
                out_tiles.append(ot)

            # ====== load x, transpose, project Q^T/K^T ======
            with tc.tile_pool(name="ps_pre", bufs=1, space="PSUM") as psp:
                xT = xw.tile([128, NKT * S], bf16, name="xT")
                xT3 = xT.rearrange("p (k s) -> p k s", k=NKT)
                for st in range(NST):
                    x_t = xw.tile([128, K], bf16, name=f"x_{st}", tag="xload", bufs=3)
                    nc.sync.dma_start(x_t[:], x_in[st * 128 : (st + 1) * 128, :])
                    for kg in range(NKT // 4):
                        tr_ps = psp.tile(
                            [128, 512], bf16, name=f"xtr_{st}_{kg}", tag="xtr", bufs=2
                        )
                        for j in range(4):
                            kk = kg * 4 + j
                            nc.tensor.transpose(
                                tr_ps[:, j * 128 : (j + 1) * 128],
                                x_t[:, kk * 128 : (kk + 1) * 128],
                                ident[:],
                            )
                        nc.vector.tensor_copy(
                            xT3[:, kg * 4 : (kg + 1) * 4, st * 128 : (st + 1) * 128],
                            tr_ps.rearrange("p (j b) -> p j b", j=4),
                        )

                wq_t = []
                wk_t = []
                wv_t = []
                for kk in range(NKT):
                    wq_kk = xw.tile([128, C], bf16, name=f"wq_{kk}", tag="wq", bufs=NKT)
                    nc.sync.dma_start(wq_kk[:], wq_in[kk * 128 : (kk + 1) * 128, :])
                    wq_t.append(wq_kk)
                    wk_kk = xw.tile([128, C], bf16, name=f"wk_{kk}", tag="wk", bufs=NKT)
                    nc.sync.dma_start(wk_kk[:], wk_in[kk * 128 : (kk + 1) * 128, :])
                    wk_t.append(wk_kk)
                    wv_kk = xw.tile([128, C], bf16, name=f"wv_{kk}", tag="wv", bufs=NKT)
                    nc.sync.dma_start(wv_kk[:], wv_in[kk * 128 : (kk + 1) * 128, :])
                    wv_t.append(wv_kk)

                for w_t, dst in ((wq_t, qt_tiles[0]), (wk_t, kt_tiles[0])):
                    for sc in range(S // 512):
                        pp = psp.tile(
                            [128, 512], f32, name=f"pj0_{sc}", tag="proj",
                            bufs=2,
                        )
                        for kk in range(NKT):
                            nc.tensor.matmul(
                                pp[:],
                                w_t[kk][:, 0:128],
                                xT3[:, kk, sc * 512 : (sc + 1) * 512],
                                start=(kk == 0),
                                stop=(kk == NKT - 1),
                            )
                        nc.vector.tensor_copy(
                            dst[:, sc * 512 : (sc + 1) * 512], pp[:]
                        )

            # ====== attention (V-proj interleaved into first loop) ======
            with tc.tile_pool(name="ps_attn", bufs=1, space="PSUM") as psa:
                tail_groups = []

                def make_vproj(st):
                    def emit():
                        va3 = v_aug[st].rearrange("p (h d) -> p h d", h=HPC)
                        vp = psa.tile(
                            [128, C], f32, name=f"vp_{st}", tag="aux", bufs=2
                        )
                        for kk in range(NKT):
                            nc.tensor.matmul(
                                vp[:],
                                xT3[:, kk, st * 128 : (st + 1) * 128],
                                wv_t[kk][:],
                                start=(kk == 0),
                                stop=(kk == NKT - 1),
                            )
                        nc.vector.tensor_copy(
                            va3[:, :, 0:HD],
                            vp.rearrange("p (h d) -> p h d", h=HPC),
                        )
                        nc.vector.tensor_copy(
                            va3[:, :, HD : HD + 1],
                            ones4.rearrange("p (h o) -> p h o", h=HPC),
                        )
                        nc.vector.tensor_copy(
                            va3[:, :, HD + 1 : HD + 2],
                            zeros4.rearrange("p (h o) -> p h o", h=HPC),
                        )
                    return emit

                def make_proj1(w_t, dst, sc):
                    def emit():
                        pp = psa.tile(
                            [128, 512], f32, name=f"pj1_{sc}", tag="aux", bufs=2
                        )
                        for kk in range(NKT):
                            nc.tensor.matmul(
                                pp[:],
                                w_t[kk][:, 128:256],
                                xT3[:, kk, sc * 512 : (sc + 1) * 512],
                                start=(kk == 0),
                                stop=(kk == NKT - 1),
                            )
                        nc.vector.tensor_copy(
                            dst[:, sc * 512 : (sc + 1) * 512], pp[:]
                        )
                    return emit

                aux_work = [make_vproj(st) for st in range(NST)]
                for w_t, dst in ((wq_t, qt_tiles[1]), (wk_t, kt_tiles[1])):
                    for sc in range(S // 512):
                        aux_work.append(make_proj1(w_t, dst, sc))
                for hp in range(HPC // 2):
                    qt = qt_tiles[hp]
                    kt = kt_tiles[hp]
                    for side in range(2):
                        dlo = side * HD
                        dhi = dlo + HD
                        h_loc = 2 * hp + side
                        for shi in range(S // SH):
                            s0 = shi * SH
                            outp = psa.tile(
                                [VAUG, SH],
                                f32,
                                name=f"outT_{h_loc}_{shi}",
                                tag="outT",
                                bufs=1,
                            )
                            for t in range(NST):
                                had_aux = bool(aux_work)
                                if had_aux:
                                    aux_work.pop(0)()
                                sc_ps = psa.tile(
                                    [128, SH],
                                    f32,
                                    name=f"sc_{h_loc}_{shi}_{t}",
                                    tag="sc",
                                    bufs=2,
                                )
                                if not had_aux:
                                    # tiny full-array matmul, result
                                    # overwritten by QK below (PE clock-gate
                                    # keep-warm; see module docstring)
                                    nc.tensor.matmul(
                                        sc_ps[:, 0:64],
                                        ident[:],
                                        ident[:, 0:64],
                                        start=True,
                                        stop=True,
                                        skip_group_check=True,
                                    )
                                for scj in range(SH // 512):
                                    nc.tensor.matmul(
                                        sc_ps[:, scj * 512 : (scj + 1) * 512],
                                        kt[dlo:dhi, t * 128 : (t + 1) * 128],
                                        qt[
                                            dlo:dhi,
                                            s0 + scj * 512 : s0 + (scj + 1) * 512,
                                        ],
                                        start=True,
                                        stop=True,
                                        skip_group_check=True,
                                    )
                                at = attn.tile(
                                    [128, SH],
                                    bf16,
                                    name=f"at_{h_loc}_{shi}_{t}",
                                    tag="at",
                                    bufs=3,
                                )
                                nc.scalar.activation(
                                    at[:], sc_ps[:], AF.Exp, scale=0.125
                                )
                                for scj in range(SH // 512):
                                    nc.tensor.matmul(
                                        outp[:, scj * 512 : (scj + 1) * 512],
                                        v_aug[t][:, h_loc * VAUG : (h_loc + 1) * VAUG],
                                        at[:, scj * 512 : (scj + 1) * 512],
                                        start=(t == 0),
                                        stop=(t == NST - 1),
                                    )
                            # free psum fast; transpose/normalize deferred
                            oT = attn.tile(
                                [80, SH],
                                bf16,
                                name=f"oT_{h_loc}_{shi}",
                                tag="oT",
                                bufs=8,
                            )
                            nc.vector.tensor_copy(oT[0:VAUG, :], outp[:])
                            tail_groups.append((h_loc, shi, oT))

                # deferred tail: DMA xbar transpose back to [s, d], then
                # normalize rows by 1/sums (col HD of transposed block)
                for h_loc, shi, oT in tail_groups:
                    trb = attn.tile(
                        [128, (SH // 128) * 80],
                        bf16,
                        name=f"trb_{h_loc}_{shi}",
                        tag="trb",
                        bufs=4,
                    )
                    trb3 = trb.rearrange("p (j c) -> p j c", j=SH // 128)
                    nc.sync.dma_start_transpose(trb3[:, :, :], oT[0:80, :])
                    for j in range(SH // 128):
                        sb = shi * (SH // 128) + j
                        rs = attn.tile(
                            [128, 1],
                            f32,
                            name=f"rs_{h_loc}_{shi}_{j}",
                            tag="rs",
                            bufs=8,
                        )
                        nc.vector.reciprocal(rs[:], trb3[:, j, HD : HD + 1])
                        nc.vector.tensor_scalar_mul(
                            out_tiles[sb][:, h_loc * HD : (h_loc + 1) * HD],
                            trb3[:, j, 0:HD],
                            rs[:],
                        )

                for sb in range(NST):
                    nc.sync.dma_start(
                        out_d[sb * 128 : (sb + 1) * 128, :], out_tiles[sb][:]
                    )

    nc.finalize()
    return nc


def _get_nc():
    if "nc" not in _CACHE:
        _CACHE["nc"] = _build_nc()
    return _CACHE["nc"]


def kernel(x, Wq, Wk, Wv):
    import ml_dtypes
    from concourse import bass_utils

    bf = ml_dtypes.bfloat16
    x = np.asarray(x, dtype=np.float32).astype(bf)
    Wq = np.asarray(Wq, dtype=np.float32).astype(bf)
    Wk = np.asarray(Wk, dtype=np.float32).astype(bf)
    Wv = np.asarray(Wv, dtype=np.float32).astype(bf)

    nc = _get_nc()
    in_maps = []
    for c in range(N_CORES):
        b = c // 4
        g = c % 4
        cols = slice(g * C, (g + 1) * C)
        in_maps.append(
            {
                "x": np.ascontiguousarray(x[b]),
                "wq": np.ascontiguousarray(Wq[:, cols]),
                "wk": np.ascontiguousarray(Wk[:, cols]),
                "wv": np.ascontiguousarray(Wv[:, cols]),
            }
        )

    res = bass_utils.run_bass_kernel_spmd(nc, in_maps, list(range(N_CORES)))
    _CACHE["last_results"] = res

    out = np.empty((B, S, RES), dtype=np.float32)
    for c in range(N_CORES):
        b = c // 4
        g = c % 4
        out[b, :, g * C : (g + 1) * C] = res.results[c]["out"]
    return out
